# revision 34
# baseline (speedup 1.0000x reference)
"""Cross-attention Trainium2 kernel, sharded over heads across 8 NeuronCores.

Core i computes head i for both batches, all in bf16 (fp8 fails the 2e-2
tolerance; bf16 lands ~5e-3):

  phase 1: q/k/v projections streamed over s.  q's DH=160 output is split
    128 ("qA") + 32 ("qB"); the qB weight columns are host-replicated 4x so
    the projection fills all 128 partitions with 4 copies of qB -- free
    replication that feeds the row-packed score matmuls.  k's tail ("kB")
    is replicated to 4 partition strips with SBUF DMAs.  v^T is built with
    PE transposes, with a ones column at 160 for the softmax denominator.

  phase 3 per q-block: scores^T = k^T q over DH=160: the 128-dim part is a
    full-array matmul per k-chunk; the 32-dim tail is packed 4 k-chunks at
    a time into disjoint 32-row strips (tile_position) running concurrently.
    exp on ScalarE -> attn^T accumulated per 128-q-chunk as [128q, 161]
    matmuls (et chunk stationary, v^T moving).  The denominator arrives as
    column 160; normalize with a per-partition reciprocal + multiply, PE
    transpose back to [d, q], and project with Wout (woB pair-packed in
    2 row strips).
Host sums the 8 partial outputs and adds bout.
"""

import sys

sys.path.insert(0, "/opt/trn_rl_repo")

import numpy as np
import ml_dtypes

import concourse.bacc as bacc
import concourse.tile as tile
from concourse import masks, mybir
from concourse.bass_utils import run_bass_kernel_spmd

HEADS = 8
DH = 160
C = 1280
B = 2
S = 4096
NQ = 512  # q-block size
NP = 512  # projection s-block size
SCALE = DH ** -0.5

_cache = {}


def _build(s=S, reps=1):
    key = (s, reps)
    if key in _cache:
        return _cache[key]
    f32, bf16 = mybir.dt.float32, mybir.dt.bfloat16
    KC = s // 128  # k chunks
    QBN = s // NQ  # q blocks
    SBN = s // NP  # projection s-blocks
    CT = C // 128  # contraction tiles for projections

    nc = bacc.Bacc("TRN2", target_bir_lowering=False, debug=False, num_devices=HEADS)
    d_x = nc.dram_tensor("x", [B, C, s], bf16, kind="ExternalInput").ap()
    d_c = nc.dram_tensor("c", [B, C, s], bf16, kind="ExternalInput").ap()
    d_msk = nc.dram_tensor("msk", [B, s], f32, kind="ExternalInput").ap()
    d_wqt = nc.dram_tensor("wqt", [C, 256], bf16, kind="ExternalInput").ap()
    d_wkt = nc.dram_tensor("wkt", [C, 128], bf16, kind="ExternalInput").ap()
    d_wvt = nc.dram_tensor("wvt", [C, 128], bf16, kind="ExternalInput").ap()
    d_w2t = nc.dram_tensor("w2t", [C, 64], bf16, kind="ExternalInput").ap()
    d_woA = nc.dram_tensor("woA", [128, C], bf16, kind="ExternalInput").ap()
    d_woB2 = nc.dram_tensor("woB2", [64, C], bf16, kind="ExternalInput").ap()
    d_out = nc.dram_tensor("out", [B, C, s], f32, kind="ExternalOutput").ap()

    x_r = d_x.rearrange("b (t p) s -> b p t s", p=128)
    c_r = d_c.rearrange("b (t p) s -> b p t s", p=128)

    with tile.TileContext(nc) as tc:
        with (
            tc.tile_pool(name="wp", bufs=1) as wp,
            tc.tile_pool(name="big", bufs=1) as big,
            tc.tile_pool(name="stream", bufs=2) as stream,
            tc.tile_pool(name="expp", bufs=6) as expp,
            tc.tile_pool(name="smal", bufs=2) as smal,
            tc.tile_pool(name="outp", bufs=3) as outp,
            tc.tile_pool(name="psS", bufs=2, space="PSUM") as psS,   # 2x2 banks
            tc.tile_pool(name="psa", bufs=1, space="PSUM") as psa,   # 4 banks
        ):
            # ---- constants / weights ----
            wqt = wp.tile([128, CT, 256], bf16)
            nc.sync.dma_start(out=wqt, in_=d_wqt.rearrange("(t p) d -> p t d", p=128))
            wkt = wp.tile([128, CT, 128], bf16)
            nc.sync.dma_start(out=wkt, in_=d_wkt.rearrange("(t p) d -> p t d", p=128))
            wvt = wp.tile([128, CT, 128], bf16)
            nc.sync.dma_start(out=wvt, in_=d_wvt.rearrange("(t p) d -> p t d", p=128))
            w2t = wp.tile([128, CT, 64], bf16)
            nc.sync.dma_start(out=w2t, in_=d_w2t.rearrange("(t p) d -> p t d", p=128))
            woA = wp.tile([128, C], bf16)
            nc.sync.dma_start(out=woA, in_=d_woA)
            woB2 = wp.tile([64, C], bf16)
            nc.sync.dma_start(out=woB2, in_=d_woB2)
            msk = wp.tile([128, B, KC], f32)
            nc.sync.dma_start(out=msk, in_=d_msk.rearrange("b (t p) -> p b t", p=128))
            # fold the additive kv-mask into v^T rows: attn num/den use
            # e^{score}*e^{mask}, so scale v (and the ones col) by e^{mask}
            em = wp.tile([128, B, KC], f32)
            nc.scalar.activation(out=em, in_=msk,
                                 func=mybir.ActivationFunctionType.Exp)
            ident = wp.tile([128, 128], bf16)
            masks.make_identity(nc, ident[:])

            import contextlib
            rep_ctx = tc.For_i(0, reps, 1) if reps > 1 else contextlib.nullcontext()
            with rep_ctx:
                for b in range(B):
                    kA = big.tile([128, s], bf16, tag="kA")
                    qA = big.tile([128, s], bf16, tag="qA")
                    kB = big.tile([128, s], bf16, tag="kB")   # 4 replica strips
                    qB = big.tile([128, s], bf16, tag="qB")   # 4 replica strips
                    vT = big.tile([128, KC, DH + 1], bf16, tag="vT")

                    # ---- phase 1: projections, streamed over s ----
                    for si in range(SBN):
                        sl = slice(NP * si, NP * si + NP)
                        kc0 = (NP * si) // 128
                        ct = stream.tile([128, CT, NP], bf16, tag="ct")
                        nc.sync.dma_start(out=ct, in_=c_r[b, :, :, sl])
                        ht = stream.tile([128, CT, NP], bf16, tag="ht")
                        nc.sync.dma_start(out=ht, in_=x_r[b, :, :, sl])

                        pk = psS.tile([128, NP], f32, tag="ps")
                        for t in range(CT):
                            nc.tensor.matmul(out=pk, lhsT=wkt[:, t, :], rhs=ct[:, t, :],
                                             start=(t == 0), stop=(t == CT - 1))
                        nc.vector.tensor_copy(out=kA[:, sl], in_=pk)

                        p2 = psS.tile([64, NP], f32, tag="ps")
                        for t in range(CT):
                            nc.tensor.matmul(out=p2, lhsT=w2t[:, t, :], rhs=ct[:, t, :],
                                             start=(t == 0), stop=(t == CT - 1))
                        st2 = stream.tile([64, NP], bf16, tag="st2")
                        nc.vector.tensor_copy(out=st2, in_=p2)
                        # kB tail -> replica strip 1 (aligned), DMA to strips 0,2,3
                        nc.sync.dma_start(out=kB[0:32, sl], in_=st2[32:64, :])
                        nc.sync.dma_start(out=kB[32:64, sl], in_=st2[32:64, :])
                        nc.sync.dma_start(out=kB[64:96, sl], in_=st2[32:64, :])
                        nc.sync.dma_start(out=kB[96:128, sl], in_=st2[32:64, :])
                        for j in range(NP // 128):
                            pt2 = psS.tile([128, 32], bf16, tag="ps")
                            nc.tensor.transpose(out=pt2,
                                                in_=st2[0:32, 128 * j:128 * j + 128],
                                                identity=ident[0:32, 0:32])
                            kci = kc0 + j
                            nc.vector.tensor_scalar_mul(vT[:, kci, 128:160], pt2[:],
                                                        em[:, b, kci:kci + 1])

                        pv = psS.tile([128, NP], f32, tag="ps")
                        for t in range(CT):
                            nc.tensor.matmul(out=pv, lhsT=wvt[:, t, :], rhs=ct[:, t, :],
                                             start=(t == 0), stop=(t == CT - 1))
                        vst = stream.tile([128, NP], bf16, tag="vst")
                        nc.vector.tensor_copy(out=vst, in_=pv)
                        for j in range(NP // 128):
                            pt1 = psS.tile([128, 128], bf16, tag="ps")
                            nc.tensor.transpose(out=pt1,
                                                in_=vst[:, 128 * j:128 * j + 128],
                                                identity=ident[:])
                            kci = kc0 + j
                            nc.vector.tensor_scalar_mul(vT[:, kci, 0:128], pt1[:],
                                                        em[:, b, kci:kci + 1])

                        pq = psS.tile([128, NP], f32, tag="ps")
                        for t in range(CT):
                            nc.tensor.matmul(out=pq, lhsT=wqt[:, t, 0:128], rhs=ht[:, t, :],
                                             start=(t == 0), stop=(t == CT - 1))
                        nc.vector.tensor_copy(out=qA[:, sl], in_=pq)

                        # qB: weight columns are host-replicated 4x, so this
                        # fills all 4 strips at full array utilization
                        pq2 = psS.tile([128, NP], f32, tag="ps")
                        for t in range(CT):
                            nc.tensor.matmul(out=pq2, lhsT=wqt[:, t, 128:256], rhs=ht[:, t, :],
                                             start=(t == 0), stop=(t == CT - 1))
                        nc.vector.tensor_copy(out=qB[:, sl], in_=pq2)

                    nc.vector.tensor_copy(out=vT[:, :, 160:161],
                                          in_=em[:, b, :])

                    # ---- phase 3: attention + output projection per q-block ----
                    for qb in range(QBN):
                        qsl = slice(NQ * qb, NQ * qb + NQ)
                        # attn^T accumulators: one bank per 128-q-chunk (PSUM
                        # accumulation groups are tracked per bank zero-region)
                        paT = [psa.tile([128, DH + 1], f32, tag=f"paT{qc}",
                                        name=f"paT{qc}")[:]
                               for qc in range(4)]
                        for g in range(KC // 4):
                            pss = [psS.tile([128, 2, NQ], f32, tag="ps", name=f"pss{j}")
                                   for j in range(2)]
                            for j in range(4):
                                kc = 4 * g + j
                                ksl = slice(128 * kc, 128 * kc + 128)
                                nc.tensor.matmul(out=pss[j // 2][:, j % 2, :],
                                                 lhsT=kA[:, ksl],
                                                 rhs=qA[:, qsl],
                                                 start=True, stop=False)
                            for j in range(4):
                                kc = 4 * g + j
                                ksl = slice(128 * kc, 128 * kc + 128)
                                p0 = 32 * j
                                nc.tensor.matmul(out=pss[j // 2][:, j % 2, :],
                                                 lhsT=kB[p0:p0 + 32, ksl],
                                                 rhs=qB[p0:p0 + 32, qsl],
                                                 start=False, stop=True,
                                                 tile_position=(p0, 0))
                            for jj in range(2):
                                et = expp.tile([128, 2, NQ], bf16, tag="et")
                                nc.scalar.activation(out=et, in_=pss[jj],
                                                     func=mybir.ActivationFunctionType.Exp,
                                                     scale=SCALE)
                                for j in range(2):
                                    kc = 4 * g + 2 * jj + j
                                    for qc in range(4):
                                        nc.tensor.matmul(
                                            out=paT[qc],
                                            lhsT=et[:, j, 128 * qc:128 * qc + 128],
                                            rhs=vT[:, kc, :],
                                            start=(kc == 0), stop=(kc == KC - 1))

                        # normalize per q-chunk, transpose back, assemble a1/a2
                        a1 = smal.tile([128, NQ], bf16, tag="a1")
                        a2 = smal.tile([64, NQ], bf16, tag="a2")  # strip 0 + rep
                        for qc in range(4):
                            rec = smal.tile([128, 1], f32, tag="rec")
                            nc.vector.reciprocal(out=rec[:],
                                                 in_=paT[qc][:, 160:161])
                            aT = smal.tile([128, DH], bf16, tag="aT")
                            nc.vector.tensor_scalar_mul(aT[:], paT[qc][:, 0:DH],
                                                        rec[:])
                            ptq = psS.tile([128, 128], bf16, tag="ps")
                            nc.tensor.transpose(out=ptq, in_=aT[:, 0:128],
                                                identity=ident[:])
                            nc.vector.tensor_copy(out=a1[:, 128 * qc:128 * qc + 128],
                                                  in_=ptq)
                            ptb = psS.tile([32, 128], bf16, tag="ps")
                            nc.tensor.transpose(out=ptb, in_=aT[:, 128:160],
                                                identity=ident[:])
                            nc.vector.tensor_copy(out=a2[0:32, 128 * qc:128 * qc + 128],
                                                  in_=ptb)
                        # replicate a2 to strip 1 for the pair-packed woB matmuls
                        nc.sync.dma_start(out=a2[32:64, :], in_=a2[0:32, :])

                        # output projection: woA full-array; woB packed in pairs
                        for oc2 in range(CT // 2):
                            poE = psS.tile([128, NQ], f32, tag="ps")
                            poO = psS.tile([128, NQ], f32, tag="ps")
                            oslE = slice(256 * oc2, 256 * oc2 + 128)
                            oslO = slice(256 * oc2 + 128, 256 * oc2 + 256)
                            nc.tensor.matmul(out=poE, lhsT=woA[:, oslE], rhs=a1[:],
                                             start=True, stop=False)
                            nc.tensor.matmul(out=poO, lhsT=woA[:, oslO], rhs=a1[:],
                                             start=True, stop=False)
                            nc.tensor.matmul(out=poE, lhsT=woB2[0:32, oslE],
                                             rhs=a2[0:32, :],
                                             start=False, stop=True,
                                             tile_position=(0, 0))
                            nc.tensor.matmul(out=poO, lhsT=woB2[32:64, oslO],
                                             rhs=a2[32:64, :],
                                             start=False, stop=True,
                                             tile_position=(32, 0))
                            otE = outp.tile([128, NQ], f32, tag="ot")
                            nc.vector.tensor_copy(out=otE, in_=poE)
                            nc.sync.dma_start(out=d_out[b, oslE, qsl], in_=otE)
                            otO = outp.tile([128, NQ], f32, tag="ot")
                            nc.vector.tensor_copy(out=otO, in_=poO)
                            nc.sync.dma_start(out=d_out[b, oslO, qsl], in_=otO)

    nc.compile()
    _cache[key] = nc
    return nc


def _prep_inputs(hidden_states, context, mask, Wq, Wk, Wv, Wout):
    bf16 = ml_dtypes.bfloat16
    x = np.ascontiguousarray(
        np.asarray(hidden_states, dtype=np.float32)[:, :, 0, :]).astype(bf16)
    c = np.ascontiguousarray(
        np.asarray(context, dtype=np.float32)[:, :, 0, :]).astype(bf16)
    msk = np.ascontiguousarray(np.asarray(mask, dtype=np.float32)[:, :, 0, 0])
    Wq = np.asarray(Wq, dtype=np.float32)
    Wk = np.asarray(Wk, dtype=np.float32)
    Wv = np.asarray(Wv, dtype=np.float32)
    Wout = np.asarray(Wout, dtype=np.float32)
    ins = []
    for h in range(HEADS):
        rows = slice(DH * h, DH * h + DH)
        wq, wk, wv = Wq[rows], Wk[rows], Wv[rows]
        wo = Wout[:, rows].T  # [160, C]
        wqt = np.concatenate([wq[0:128].T] + [wq[128:160].T] * 4, axis=1)  # [C,256]
        ins.append({
            "x": x,
            "c": c,
            "msk": msk,
            "wqt": np.ascontiguousarray(wqt).astype(bf16),
            "wkt": np.ascontiguousarray(wk[0:128].T).astype(bf16),
            "wvt": np.ascontiguousarray(wv[0:128].T).astype(bf16),
            "w2t": np.ascontiguousarray(
                np.concatenate([wv[128:160], wk[128:160]], axis=0).T).astype(bf16),
            "woA": np.ascontiguousarray(wo[0:128]).astype(bf16),
            "woB2": np.ascontiguousarray(
                np.concatenate([wo[128:160], wo[128:160]], axis=0)).astype(bf16),
        })
    return ins


def kernel(hidden_states, context, mask, Wq, Wk, Wv, Wout, bout):
    nc = _build()
    ins = _prep_inputs(hidden_states, context, mask, Wq, Wk, Wv, Wout)
    res = run_bass_kernel_spmd(nc, ins, core_ids=list(range(HEADS)))
    total = res.results[0]["out"].astype(np.float32)
    for h in range(1, HEADS):
        total = total + res.results[h]["out"]
    total = total + np.asarray(bout, dtype=np.float32)[None, :, None]
    return total[:, :, None, :].astype(np.float32)
